# revision 1
# baseline (speedup 1.0000x reference)
"""ChildSum TreeLSTM on a complete binary tree — Trainium2 Bass kernel.

Sharding: data-parallel over the batch of trees (B=8 -> 8 NeuronCores, one
tree per core).  Weights are replicated.  Everything on-chip lives in
transposed [feature, node] layout so x streams straight into the PE as the
moving operand; the host pre-transposes x per core and casts to bf16.

Tree is processed bottom-up level by level, in column chunks of <=256 nodes,
emitted in post-order over the chunk tree so the live h/c frontier stays
small and the Tile scheduler can pipeline PE/ACT/DVE/DMA across chunks.
"""

import sys

sys.path.insert(0, "/opt/trn_rl_repo")

import numpy as np
import ml_dtypes

import bass_rust
import concourse.bass as bass
import concourse.mybir as mybir
from concourse.tile import TileContext
from concourse.bass_utils import run_bass_kernel_spmd


def _split_waits(nc, compute_limit=1, dma_limit=1):
    """Walrus in this container accepts few fused sync-waits per instruction
    (1 for DMA descriptors, ~2 for compute).  Move excess waits onto
    same-engine nop instructions inserted right before the offender."""
    eng_map = {
        mybir.EngineType.DVE: nc.vector,
        mybir.EngineType.Activation: nc.scalar,
        mybir.EngineType.PE: nc.tensor,
        mybir.EngineType.Pool: nc.gpsimd,
        mybir.EngineType.SP: nc.sync,
    }

    active_block = [None]

    def make_nop(engine):
        bi = eng_map[engine].nop()
        inst = bi.ins
        ab = active_block[0]
        if ab is not None and ab.instructions and ab.instructions[-1] is inst:
            ab.instructions.pop()
            return inst
        for f in nc.m.functions:
            for b in f.blocks:
                if b.instructions and b.instructions[-1] is inst:
                    b.instructions.pop()
                    active_block[0] = b
                    return inst
        raise RuntimeError("nop not found")

    dma_types = {"InstDMACopy", "InstDMA", "InstDmaTransposeAnt", "InstDrain"}
    for f in nc.m.functions:
        for b in f.blocks:
            new = []
            for inst in list(b.instructions):
                si = inst.sync_info
                waits = list(si.on_wait) if si is not None and si.on_wait else []
                tname = type(inst).__name__
                eng = getattr(inst, "engine", None)
                limit = dma_limit if tname in dma_types else compute_limit
                nop_limit = dma_limit if eng == mybir.EngineType.SP else compute_limit
                if len(waits) > limit and eng in eng_map:
                    excess, keep = waits[:-limit] if limit else waits, waits[-limit:] if limit else []
                    for i0 in range(0, len(excess), nop_limit):
                        nop = make_nop(eng)
                        nop.sync_info = bass_rust.SyncInfo(
                            on_wait=excess[i0:i0 + nop_limit], on_update=[]
                        )
                        new.append(nop)
                    inst.sync_info = bass_rust.SyncInfo(
                        on_wait=keep, on_update=list(si.on_update) if si.on_update else []
                    )
                new.append(inst)
            b.instructions.clear()
            b.instructions.extend(new)

P = 128
D_IN = 256
D_H = 256
BF16 = mybir.dt.bfloat16
F32 = mybir.dt.float32
AF = mybir.ActivationFunctionType
ALU = mybir.AluOpType

_NC_CACHE = {}


def build_nc(L):
    """Build the single-core SPMD Bass program for a tree with L leaves."""
    D = int(np.log2(L))
    assert 2**D == L
    N = 2 * L - 1

    nc = bass.Bass()

    xT = nc.dram_tensor("xT", [D_IN, N], BF16, kind="ExternalInput")
    w_iou_d = nc.dram_tensor("w_iou", [D_IN, 3 * D_H], BF16, kind="ExternalInput")
    u_iou_d = nc.dram_tensor("u_iou", [D_H, 3 * D_H], BF16, kind="ExternalInput")
    w_f_d = nc.dram_tensor("w_f", [D_IN, D_H], BF16, kind="ExternalInput")
    u_f_d = nc.dram_tensor("u_f", [D_H, D_H], BF16, kind="ExternalInput")
    b_iou_d = nc.dram_tensor("b_iou_t", [P, 6], F32, kind="ExternalInput")
    b_f_d = nc.dram_tensor("b_f_t", [P, 2], F32, kind="ExternalInput")
    out_d = nc.dram_tensor("out", [2, D_H], F32, kind="ExternalOutput")

    C_LEAF = min(256, L)

    def n_chunks(lvl):
        n = 2**lvl
        if lvl == D:
            return L // C_LEAF
        return n // min(n, 256)

    with TileContext(nc) as tc:
        with (
            tc.tile_pool(name="const", bufs=1) as cpool,
            tc.tile_pool(name="xa", bufs=4) as xpool,
            tc.tile_pool(name="h", bufs=8) as hpool,
            tc.tile_pool(name="c", bufs=8) as cfpool,
            tc.tile_pool(name="g", bufs=3) as gpool,
            tc.tile_pool(name="ps", bufs=8, space="PSUM") as pspool,
        ):
            # ---- replicated weights / biases into SBUF ----
            w_iou = [cpool.tile([P, 3 * D_H], BF16, tag=f"w_iou{k}", name=f"w_iou{k}") for k in range(2)]
            u_iou = [cpool.tile([P, 3 * D_H], BF16, tag=f"u_iou{k}", name=f"u_iou{k}") for k in range(2)]
            w_f = [cpool.tile([P, D_H], BF16, tag=f"w_f{k}", name=f"w_f{k}") for k in range(2)]
            u_f = [cpool.tile([P, D_H], BF16, tag=f"u_f{k}", name=f"u_f{k}") for k in range(2)]
            for k in range(2):
                nc.gpsimd.dma_start(out=w_iou[k], in_=w_iou_d[k * P:(k + 1) * P, :])
                nc.gpsimd.dma_start(out=u_iou[k], in_=u_iou_d[k * P:(k + 1) * P, :])
                nc.gpsimd.dma_start(out=w_f[k], in_=w_f_d[k * P:(k + 1) * P, :])
                nc.gpsimd.dma_start(out=u_f[k], in_=u_f_d[k * P:(k + 1) * P, :])
            b_iou = cpool.tile([P, 6], F32, tag="b_iou", name="b_iou")
            b_f = cpool.tile([P, 2], F32, tag="b_f", name="b_f")
            nc.gpsimd.dma_start(out=b_iou, in_=b_iou_d[:, :])
            nc.gpsimd.dma_start(out=b_f, in_=b_f_d[:, :])

            h_tiles = {}  # (lvl, tile_idx, k) -> AP   bf16 [P, S]
            c_tiles = {}  # (lvl, tile_idx, k) -> AP   f32  [P, S]
            root = {}

            def load_x(lvl, col0, C):
                off = 2**lvl - 1
                xa = [xpool.tile([P, C], BF16, tag=f"xa{k}", name=f"xa{k}") for k in range(2)]
                for k in range(2):
                    nc.sync.dma_start(
                        out=xa[k],
                        in_=xT[k * P:(k + 1) * P, off + col0: off + col0 + C],
                    )
                return xa

            def alloc_hc(lvl, j, C, fp32_h=False):
                n = 2**lvl
                S = min(n, 512)
                ti, co = (j * C) // S, (j * C) % S
                if co == 0:
                    for k in range(2):
                        h_tiles[(lvl, ti, k)] = hpool.tile(
                            [P, S], F32 if fp32_h else BF16, tag=f"h{k}", name=f"h{k}"
                        )
                        c_tiles[(lvl, ti, k)] = cfpool.tile([P, S], F32, tag=f"c{k}", name=f"c{k}")
                hs = [h_tiles[(lvl, ti, k)][:, co:co + C] for k in range(2)]
                cs = [c_tiles[(lvl, ti, k)][:, co:co + C] for k in range(2)]
                return hs, cs

            def emit_leaf(j):
                C = C_LEAF
                xa = load_x(D, j * C, C)
                h_sl, c_sl = alloc_hc(D, j, C)
                gi, gu, go, gtc = ({} for _ in range(4))
                for k in range(2):
                    for g, name, func, store in (
                        (0, "i", AF.Sigmoid, gi),
                        (1, "o", AF.Sigmoid, go),
                        (2, "u", AF.Tanh, gu),
                    ):
                        ps = pspool.tile([P, C], F32, tag="ps", name="ps")
                        lhs0 = w_iou[0][:, g * D_H + k * P: g * D_H + (k + 1) * P]
                        lhs1 = w_iou[1][:, g * D_H + k * P: g * D_H + (k + 1) * P]
                        nc.tensor.matmul(out=ps, lhsT=lhs0, rhs=xa[0], start=True, stop=False)
                        nc.tensor.matmul(out=ps, lhsT=lhs1, rhs=xa[1], start=False, stop=True)
                        t = gpool.tile([P, C], F32, tag=f"g{name}{k}", name=f"g{name}{k}")
                        nc.scalar.activation(t, ps, func, bias=b_iou[:, 2 * g + k: 2 * g + k + 1])
                        store[k] = t
                for k in range(2):
                    # c = sig(i) * tanh(u)
                    nc.vector.tensor_tensor(c_sl[k], gi[k], gu[k], ALU.mult)
                    t = gpool.tile([P, C], F32, tag=f"gtc{k}", name=f"gtc{k}")
                    nc.scalar.activation(t, c_sl[k], AF.Tanh)
                    gtc[k] = t
                for k in range(2):
                    nc.vector.tensor_tensor(h_sl[k], go[k], gtc[k], ALU.mult)

            def emit_internal(lvl, j):
                n = 2**lvl
                C = min(n, 256)
                xa = load_x(lvl, j * C, C)
                # children: tile j of level lvl+1 holds cols [2jC, 2jC+2C)
                hch = [h_tiles[(lvl + 1, j, k)] for k in range(2)]
                cch = [c_tiles[(lvl + 1, j, k)] for k in range(2)]
                h_sl, c_sl = alloc_hc(lvl, j, C, fp32_h=(lvl == 0))

                # child-sum of h (bf16, SBUF-only)
                hs = []
                for k in range(2):
                    t = gpool.tile([P, C], BF16, tag=f"hs{k}", name=f"hs{k}")
                    nc.vector.tensor_tensor(t, hch[k][:, 0::2], hch[k][:, 1::2], ALU.add)
                    hs.append(t)

                # iou = W_iou.x + U_iou.h_sum  (PSUM accumulation)
                gi, gu, go = {}, {}, {}
                for g, name, func, store in (
                    (0, "i", AF.Sigmoid, gi),
                    (1, "o", AF.Sigmoid, go),
                    (2, "u", AF.Tanh, gu),
                ):
                    ps = pspool.tile([P, 2 * C], F32, tag="ps", name="ps")
                    for k in range(2):
                        o = ps[:, k * C:(k + 1) * C]
                        c0 = g * D_H + k * P
                        nc.tensor.matmul(out=o, lhsT=w_iou[0][:, c0:c0 + P], rhs=xa[0], start=True, stop=False)
                        nc.tensor.matmul(out=o, lhsT=w_iou[1][:, c0:c0 + P], rhs=xa[1], start=False, stop=False)
                        nc.tensor.matmul(out=o, lhsT=u_iou[0][:, c0:c0 + P], rhs=hs[0], start=False, stop=False)
                        nc.tensor.matmul(out=o, lhsT=u_iou[1][:, c0:c0 + P], rhs=hs[1], start=False, stop=True)
                    for k in range(2):
                        t = gpool.tile([P, C], F32, tag=f"g{name}{k}", name=f"g{name}{k}")
                        nc.scalar.activation(
                            t, ps[:, k * C:(k + 1) * C], func,
                            bias=b_iou[:, 2 * g + k: 2 * g + k + 1],
                        )
                        store[k] = t

                # f_pre = U_f.h_children + W_f.x (x broadcast to both children)
                xb = [
                    xa[k].rearrange("p (c one) -> p c one", one=1).broadcast_to((P, C, 2))
                    for k in range(2)
                ]
                f = {}
                for k in range(2):
                    ps_f = pspool.tile([P, 2 * C], F32, tag="ps", name="ps")
                    nc.tensor.matmul(out=ps_f, lhsT=u_f[0][:, k * P:(k + 1) * P], rhs=hch[0], start=True, stop=False)
                    nc.tensor.matmul(out=ps_f, lhsT=u_f[1][:, k * P:(k + 1) * P], rhs=hch[1], start=False, stop=False)
                    nc.tensor.matmul(out=ps_f, lhsT=w_f[0][:, k * P:(k + 1) * P], rhs=xb[0], start=False, stop=False)
                    nc.tensor.matmul(out=ps_f, lhsT=w_f[1][:, k * P:(k + 1) * P], rhs=xb[1], start=False, stop=True)
                    t = gpool.tile([P, 2 * C], F32, tag=f"f{k}", name=f"f{k}")
                    nc.scalar.activation(t, ps_f, AF.Sigmoid, bias=b_f[:, k:k + 1])
                    f[k] = t

                gtc = {}
                for k in range(2):
                    # fc = f * c_children ; csum = pairwise sum ; c = i*u + csum
                    fc = gpool.tile([P, 2 * C], F32, tag=f"fc{k}", name=f"fc{k}")
                    nc.vector.tensor_tensor(fc, f[k], cch[k], ALU.mult)
                    cs = gpool.tile([P, C], F32, tag=f"cs{k}", name=f"cs{k}")
                    nc.vector.tensor_tensor(cs, fc[:, 0::2], fc[:, 1::2], ALU.add)
                    iu = gpool.tile([P, C], F32, tag=f"iu{k}", name=f"iu{k}")
                    nc.vector.tensor_tensor(iu, gi[k], gu[k], ALU.mult)
                    nc.vector.tensor_tensor(c_sl[k], iu, cs, ALU.add)
                    t = gpool.tile([P, C], F32, tag=f"gtc{k}", name=f"gtc{k}")
                    nc.scalar.activation(t, c_sl[k], AF.Tanh)
                    gtc[k] = t
                for k in range(2):
                    nc.vector.tensor_tensor(h_sl[k], go[k], gtc[k], ALU.mult)

            def rec(lvl, j):
                if lvl == D:
                    emit_leaf(j)
                    return
                ratio = n_chunks(lvl + 1) // n_chunks(lvl)
                if ratio == 2:
                    rec(lvl + 1, 2 * j)
                    rec(lvl + 1, 2 * j + 1)
                else:
                    rec(lvl + 1, j)
                emit_internal(lvl, j)

            rec(0, 0)

            # root h (fp32) and c -> out
            for k in range(2):
                nc.sync.dma_start(
                    out=out_d[0:1, k * P:(k + 1) * P], in_=h_tiles[(0, 0, k)][:, 0:1]
                )
                nc.sync.dma_start(
                    out=out_d[1:2, k * P:(k + 1) * P], in_=c_tiles[(0, 0, k)][:, 0:1]
                )

    _split_waits(nc)
    return nc


def get_nc(L):
    if L not in _NC_CACHE:
        _NC_CACHE[L] = build_nc(L)
    return _NC_CACHE[L]


def prepare_in_maps(x, W_iou, b_iou, U_iou, W_f, b_f, U_f):
    bf16 = ml_dtypes.bfloat16
    B = x.shape[0]
    common = {
        "w_iou": np.asarray(W_iou, dtype=bf16),
        "u_iou": np.asarray(U_iou, dtype=bf16),
        "w_f": np.asarray(W_f, dtype=bf16),
        "u_f": np.asarray(U_f, dtype=bf16),
        "b_iou_t": np.ascontiguousarray(
            np.asarray(b_iou, dtype=np.float32).reshape(6, P).T
        ),
        "b_f_t": np.ascontiguousarray(
            np.asarray(b_f, dtype=np.float32).reshape(2, P).T
        ),
    }
    in_maps = []
    for b in range(B):
        xTb = np.ascontiguousarray(np.asarray(x[b], dtype=np.float32).T).astype(bf16)
        in_maps.append({"xT": xTb, **common})
    return in_maps


def run(inputs, trace=False):
    x = np.asarray(inputs["x"])
    B, N, _ = x.shape
    L = (N + 1) // 2
    nc = get_nc(L)
    in_maps = prepare_in_maps(
        x, inputs["W_iou"], inputs["b_iou"], inputs["U_iou"],
        inputs["W_f"], inputs["b_f"], inputs["U_f"],
    )
    res = run_bass_kernel_spmd(nc, in_maps, core_ids=list(range(B)), trace=trace)
    out = np.zeros((B, 2 * D_H), dtype=np.float32)
    for b in range(B):
        o = np.asarray(res.results[b]["out"], dtype=np.float32)
        out[b, :D_H] = o[0]
        out[b, D_H:] = o[1]
    return out, res


def kernel(**inputs):
    out, _ = run(inputs, trace=False)
    return out

